# revision 37
# baseline (speedup 1.0000x reference)
"""CrossViewAttention Trainium2 kernel (v2).

Sharding: Q=2500 query positions across 8 cores (padded to 2560 = 8*320).
Softmax is over NK which stays local per core -> no collectives.

Per-core layout is fully "transposed": features on partitions, queries on
the free dim.  Host pre-normalizes q/k/v rows (LN folded into the
projection weights) and pre-multiplies W_logits*vis into a single mask.

Attention inner loop per nk-tile t (50 tiles of 128 keys):
  - 4 QK^T matmuls, one per head (contract dim 32), issued to distinct
    PE row groups via tile_position -> they run concurrently.
  - two DVE tensor_muls apply the combined W*vis mask (head pairs, so
    the single-buffered pl PSUM tiles free up early for the next QK).
  - one ACT exp() over all four heads [128, 1280].
  - 4 attn@V matmuls accumulate [33,320] per head; the 33rd row of V is
    ones and yields the softmax denominator for free.
QK^T for tile t+1 is emitted before attn@V for tile t so the PE FIFO
never stalls the DVE multiply chain.  A PE warm-up burst at the start
un-throttles the HAM clock gate during the input-DMA window.

Epilogue: softmax denominators are shifted to partition 0 by an
SBUF-to-SBUF DMA (the custom fast-reciprocal DVE op only works at
partition base 0), one reciprocal + per-head K=1 broadcast matmuls,
projection accumulate, skip add, LN -> MLP(gelu) -> LN.  LN uses
moment matmuls against a 1/128-scaled ones column, rstd = 1/sqrt via
ACT Sqrt + fast reciprocal, and the affine (g, b fold) is built as PE
outer products.  Activation functions are chosen so only the
exp/sqrt/gelu table sets are ever loaded (no ln/exp thrash).
"""

import sys

if "/opt/trn_rl_repo" not in sys.path:
    sys.path.insert(0, "/opt/trn_rl_repo")

import numpy as np
import ml_dtypes

import concourse.bass as bass
import concourse.bacc as bacc_mod
import concourse.mybir as mybir
from concourse.tile import TileContext

# problem constants (hardcoded per harness contract)
HEADS = 4
DH = 32
D = 128
EPS = 1e-5
HB = WB = 50
Q = HB * WB            # 2500
NVIEW, KH, KW = 6, 24, 44
NK = NVIEW * KH * KW   # 6336
NCORES = 8
QC = 320               # queries per core (Q padded to 2560)
QPAD = NCORES * QC
NKP = 6400             # NK padded to 50*128
NKT = NKP // 128       # 50 nk tiles
SCALE = DH ** -0.5

F32 = mybir.dt.float32
BF16 = mybir.dt.bfloat16
AF = mybir.ActivationFunctionType
ALU = mybir.AluOpType

_CACHE = {}


def _ln_cols(nc, pools, x, g2_lhsT, out, ones_cols, sr2):
    """LayerNorm of x [128, QC] f32 SBUF over the PARTITION dim.

    Stats come from ones-matmuls (ones scaled by 1/128 so the sums are the
    moments directly); rstd = exp(-0.5*ln(var+eps)); the affine
    (x - m)*rstd*g + b is applied as x*pA + pC where pA = g (x) rstd and
    pC = g (x) (-m*rstd) + b (x) 1 are built as PE outer products.
    """
    work, epi = pools
    ones_col, ones_colf = ones_cols
    sq = work.tile([D, QC], BF16, tag="lnsq")
    nc.scalar.activation(sq, x, AF.Square)
    s1 = epi.tile([1, QC], F32, tag="lns1")
    nc.tensor.matmul(s1, ones_colf, x, start=True, stop=True)
    s2 = epi.tile([1, QC], F32, tag="lns2")
    nc.tensor.matmul(s2, ones_col, sq, start=True, stop=True)
    ms = work.tile([1, QC], F32, tag="lnms")
    nc.scalar.activation(ms, s1, AF.Square)
    var = work.tile([1, QC], F32, tag="lnvar")
    nc.vector.tensor_tensor(out=var, in0=s2, in1=ms,
                            op=ALU.subtract)
    std = work.tile([1, QC], F32, tag="lnstd")
    nc.scalar.activation(std, var, AF.Sqrt, bias=EPS)
    rstd = work.tile([1, QC], F32, tag="lnrstd")
    nc.vector.reciprocal_approx_fast(out=rstd, in_=std)
    # sr2 row0 = -m*rstd, row1 = 1.0 (preset by caller)
    nc.vector.scalar_tensor_tensor(out=sr2[0:1, :], in0=s1,
                                   scalar=-1.0, in1=rstd,
                                   op0=ALU.mult, op1=ALU.mult)
    pA = epi.tile([D, QC], F32, tag="lnpA")
    nc.tensor.matmul(pA, g2_lhsT[0:1, :], rstd, start=True, stop=True)
    pC = epi.tile([D, QC], F32, tag="lnpC")
    nc.tensor.matmul(pC, g2_lhsT[0:2, :], sr2, start=True, stop=True)
    t1 = work.tile([D, QC], F32, tag="lnt1")
    nc.vector.tensor_mul(out=t1, in0=x, in1=pA)
    nc.vector.tensor_add(out=out, in0=t1, in1=pC)


def _build():
    if "nc" in _CACHE:
        return _CACHE["nc"]
    nc = bacc_mod.Bacc()

    # ---- I/O ----
    qTn = nc.dram_tensor("qTn", [D, QC], BF16, kind="ExternalInput")
    kTn = nc.dram_tensor("kTn", [D, NKP], BF16, kind="ExternalInput")
    vTn = nc.dram_tensor("vTn", [D, NKP], BF16, kind="ExternalInput")
    Wt = nc.dram_tensor("Wt", [128, NKT, QC], BF16, kind="ExternalInput")
    skipT = nc.dram_tensor("skipT", [D, QC], F32, kind="ExternalInput")
    # packed weights: [wq|wk|wv|w1(256)|w2(256)|wproj(512, rows 0:32)]
    wpackB = nc.dram_tensor("wpackB", [D, 1408], BF16, kind="ExternalInput")
    # packed f32 params: bq|bk|bproj|b2 (cols 0:4), b1m (4:6),
    # gpre (rows 0:2, cols 6:134), gpost (rows 0:2, cols 134:262)
    wpackF = nc.dram_tensor("wpackF", [D, 262], F32, kind="ExternalInput")
    outT = nc.dram_tensor("outT", [D, QC], F32, kind="ExternalOutput")

    with TileContext(nc) as tc:
        with tc.tile_pool(name="const", bufs=1) as cpool, \
             tc.tile_pool(name="big", bufs=1) as bigpool, \
             tc.tile_pool(name="work", bufs=3) as work, \
             tc.tile_pool(name="io", bufs=1) as io:

            # ---- constants ----
            zero_c = cpool.tile([128, 1], F32)
            nc.any.memset(zero_c, 0.0)
            nc.const_aps.aps[(F32, 0.0)] = zero_c[:]
            eps_c = cpool.tile([128, 1], F32)
            nc.any.memset(eps_c, EPS)
            nc.const_aps.aps[(F32, EPS)] = eps_c[:]
            ones_col = cpool.tile([128, 1], BF16)
            nc.any.memset(ones_col, 1.0 / 128.0)
            ones_colf = cpool.tile([128, 1], F32)
            nc.any.memset(ones_colf, 1.0 / 128.0)
            ones32bh = cpool.tile([1, 32], BF16)
            nc.any.memset(ones32bh, 1.0)
            identw = cpool.tile([D, D], BF16)
            nc.any.memset(identw, 0.5)

            wpB = cpool.tile([D, 1408], BF16)
            wpF = cpool.tile([D, 262], F32)
            wq_s = wpB[:, 0:128]
            wk_s = wpB[:, 128:256]
            wv_s = wpB[:, 256:384]
            w1_s = wpB[:, 384:640]
            w2_s = wpB[:, 640:896].rearrange("p (j d) -> p j d", j=2)
            wproj_s = wpB[0:DH, 896:1408].rearrange("p (h d) -> p h d",
                                                    h=HEADS)
            bq_s = wpF[:, 0:1]
            bk_s = wpF[:, 1:2]
            bproj_s = wpF[:, 2:3]
            b2_s = wpF[:, 3:4]
            b1_s = wpF[:, 4:6]
            gpre_s = wpF[0:2, 6:134]
            gpost_s = wpF[0:2, 134:262]

            # ---- resident tensors ----
            qTn_s = bigpool.tile([D, QC], BF16)
            skip_s = bigpool.tile([D, QC], F32)
            KCH = 512
            kTn_s = bigpool.tile([D, NKP], BF16)
            vTn_s = bigpool.tile([D, NKP], BF16)
            for c0 in range(0, NKP, 1600):
                nc.scalar.dma_start(kTn_s[:, c0:c0 + 1600],
                                    kTn[:, c0:c0 + 1600])
            for c0 in range(0, NKP, 1600):
                nc.gpsimd.dma_start(vTn_s[:, c0:c0 + 1600],
                                    vTn[:, c0:c0 + 1600])
            nc.scalar.dma_start(wpB, wpackB[...])
            nc.scalar.dma_start(wpF, wpackF[...])
            nc.scalar.dma_start(qTn_s, qTn[...])
            nc.gpsimd.dma_start(skip_s, skipT[...])

            kf = bigpool.tile([D, NKT, 128], BF16)
            vf = bigpool.tile([128, NKT, HEADS, DH + 1], BF16)
            qf = bigpool.tile([D, QC], BF16)
            Wsb = bigpool.tile([128, NKT, QC], BF16)
            WCH = 5
            for t0 in range(0, NKT, WCH):
                nc.sync.dma_start(Wsb[:, t0:t0 + WCH, :],
                                  Wt[:, t0:t0 + WCH, :])

            # ones column of V (softmax denominator); zero the k-padding rows
            nc.any.memset(vf[:, :, :, DH], 1.0)

            # ---- prep: projections ----
            with tc.tile_pool(name="psum_prep", bufs=2, space="PSUM") as ppre:
                # PE warm-up: dense matmuls from t~0 un-throttle the HAM
                # clock gate (1.2 -> 2.4 GHz) and bridge the input-DMA wait.
                for _ in range(30):
                    warm = ppre.tile([D, KCH], F32, tag="pk", name="warm")
                    nc.tensor.matmul(warm[:, :D], identw, identw, start=True,
                                     stop=True)
                pq = ppre.tile([D, KCH], F32, tag="pk", name="pq")
                nc.tensor.matmul(pq[:, :QC], wq_s, qTn_s, start=True,
                                 stop=True)
                nc.scalar.activation(qf, pq[:, :QC], AF.Identity, bias=bq_s)

                for i, c0 in enumerate(range(0, NKP, KCH)):
                    ce = min(c0 + KCH, NKP)
                    nt = (ce - c0) // 128
                    pk = ppre.tile([D, KCH], F32, tag="pk")
                    nc.tensor.matmul(pk[:, :ce - c0], wk_s, kTn_s[:, c0:ce],
                                     start=True, stop=True)
                    dst = kf[:, 4 * i:4 * i + nt, :]
                    if i % 2 == 0:
                        nc.scalar.activation(dst, pk[:, :ce - c0], AF.Identity,
                                             bias=bk_s)
                    else:
                        nc.vector.tensor_scalar(out=dst, in0=pk[:, :ce - c0],
                                                scalar1=bk_s,
                                                scalar2=None, op0=ALU.add)

                for i, t0 in enumerate(range(0, NKT, 4)):
                    nt = min(4, NKT - t0)
                    pv = ppre.tile([128, 4, 128], F32, tag="pv")
                    for j in range(nt):
                        t = t0 + j
                        nc.tensor.matmul(pv[:, j, :],
                                         vTn_s[:, t * 128:(t + 1) * 128],
                                         wv_s, start=True, stop=True)
                    src = pv[:, :nt, :].rearrange("p t (h e) -> p t h e",
                                                  h=HEADS)
                    dst = vf[:, t0:t0 + nt, :, :DH]
                    if i % 2 == 0:
                        nc.vector.tensor_copy(out=dst, in_=src)
                    else:
                        nc.scalar.activation(dst, src, AF.Identity)

                # zero v-values AND ones-row at the 64 padded key rows
                nc.any.memset(vf[64:128, NKT - 1, :, :], 0.0)

            # ---- attention ----
            # Software-pipelined: QK^T for t+1 is emitted BEFORE attn@V for
            # t so the PE FIFO never blocks the DVE multiply chain.
            with tc.tile_pool(name="psum_po", bufs=1, space="PSUM") as pop:
                po = [pop.tile([DH + 1, QC], F32, tag=f"po{h}",
                               name=f"po{h}")
                      for h in range(HEADS)]
                with tc.tile_pool(name="psum_pl", bufs=1, space="PSUM") as plp, \
                     tc.tile_pool(name="attw", bufs=2) as attw:
                    def qk(t):
                        plA = plp.tile([128, 2, 512], F32, tag="plA",
                                       name="plA")
                        plB = plp.tile([128, 2, 512], F32, tag="plB",
                                       name="plB")
                        for h in range(HEADS):
                            hb = 32 * h
                            dst = (plA, plB)[h // 2][:, h % 2, :QC]
                            nc.tensor.matmul(dst,
                                             kf[hb:hb + 32, t, :],
                                             qf[hb:hb + 32, :],
                                             start=True, stop=True,
                                             tile_position=(hb, 0))
                        return plA, plB

                    plA, plB = qk(0)
                    for tp in range(NKT // 2):
                        t0, t1 = 2 * tp, 2 * tp + 1
                        em = attw.tile([128, 2, HEADS, QC], BF16, tag="em")
                        ee = attw.tile([128, 2, HEADS, QC], BF16, tag="ee")
                        wbc = Wsb[:, t0, None, :].to_broadcast((128, 2, QC))
                        nc.vector.tensor_mul(out=em[:, 0, 0:2, :],
                                             in0=plA[:, :, :QC], in1=wbc)
                        nc.vector.tensor_mul(out=em[:, 0, 2:4, :],
                                             in0=plB[:, :, :QC], in1=wbc)
                        plA, plB = qk(t1)
                        wbc = Wsb[:, t1, None, :].to_broadcast((128, 2, QC))
                        nc.vector.tensor_mul(out=em[:, 1, 0:2, :],
                                             in0=plA[:, :, :QC], in1=wbc)
                        nc.vector.tensor_mul(out=em[:, 1, 2:4, :],
                                             in0=plB[:, :, :QC], in1=wbc)
                        nc.scalar.activation(ee, em, AF.Exp)
                        if t1 + 1 < NKT:
                            plA, plB = qk(t1 + 1)
                        for tt, t in enumerate((t0, t1)):
                            for h in range(HEADS):
                                nc.tensor.matmul(po[h], vf[:, t, h, :],
                                                 ee[:, tt, h, :],
                                                 start=(t == 0),
                                                 stop=(t == NKT - 1))

                # ---- head normalize + projection accumulate ----
                with tc.tile_pool(name="psum_epi1", bufs=1, space="PSUM") as ep1:
                    den4 = io.tile([33, HEADS, QC], F32, tag="den4")
                    pz = ep1.tile([D, QC], F32, tag="pz")
                    for h in range(HEADS):
                        nc.scalar.activation(den4[DH:DH + 1, h, :],
                                             po[h][DH:DH + 1, :], AF.Copy)
                    # partition-shift 32 -> 0 via SBUF-to-SBUF DMA, then one
                    # fast reciprocal (the custom DVE op needs base 0)
                    den0 = io.tile([1, HEADS * QC], F32, tag="den0")
                    nc.sync.dma_start(
                        den0, den4[DH:DH + 1, :, :].rearrange("p h q -> p (h q)"))
                    rcp0 = io.tile([1, HEADS * QC], F32, tag="rcp0")
                    nc.vector.reciprocal_approx_fast(out=rcp0, in_=den0)
                    rcp0b = io.tile([1, HEADS * QC], BF16, tag="rcp0b")
                    nc.vector.tensor_copy(out=rcp0b, in_=rcp0)
                    rba = work.tile([DH, HEADS, QC], BF16, tag="rba")
                    for h in range(HEADS):
                        prh = ep1.tile([DH, 512], F32, tag="prh", bufs=2,
                                       name="prh")
                        nc.tensor.matmul(prh[:, :QC], ones32bh[0:1, :],
                                         rcp0b[:, h * QC:(h + 1) * QC],
                                         start=True, stop=True)
                        nc.scalar.activation(rba[:, h, :], prh[:, :QC],
                                             AF.Copy)
                    for h in range(HEADS):
                        onh = work.tile([DH, QC], BF16, tag="onh", name="onh")
                        nc.vector.tensor_mul(out=onh, in0=po[h][:DH, :],
                                             in1=rba[:, h, :])
                        nc.tensor.matmul(pz, wproj_s[:, h, :], onh,
                                         start=(h == 0),
                                         stop=(h == HEADS - 1))

                    # z = pz + bproj + skip
                    z = io.tile([D, QC], F32, tag="z")
                    nc.vector.scalar_tensor_tensor(out=z, in0=pz,
                                                   scalar=bproj_s,
                                                   in1=skip_s,
                                                   op0=ALU.add, op1=ALU.add)

            # ---- LN -> MLP -> LN ----
            with tc.tile_pool(name="psum_epi2", bufs=1, space="PSUM") as ep2:
                sr2 = io.tile([2, QC], F32, tag="sr2")
                nc.any.memset(sr2, 1.0)
                zf = io.tile([D, QC], F32, tag="zf")
                _ln_cols(nc, (work, ep2), z, gpre_s, zf,
                         (ones_col, ones_colf), sr2)
                zfb = io.tile([D, QC], BF16, tag="zfb")
                nc.vector.tensor_copy(out=zfb, in_=zf)

                h1 = io.tile([D, 2, QC], BF16, tag="h1")
                for j in range(2):
                    ph = ep2.tile([D, QC], F32, tag="ph", bufs=2)
                    nc.tensor.matmul(ph, w1_s[:, D * j:D * (j + 1)], zfb,
                                     start=True, stop=True)
                    nc.scalar.activation(h1[:, j, :], ph, AF.Gelu,
                                         bias=b1_s[:, j:j + 1])
                pm = ep2.tile([D, QC], F32, tag="pm")
                nc.tensor.matmul(pm, w2_s[:, 0, :], h1[:, 0, :],
                                 start=True, stop=False)
                nc.tensor.matmul(pm, w2_s[:, 1, :], h1[:, 1, :],
                                 start=False, stop=True)
                z3 = io.tile([D, QC], F32, tag="z3")
                nc.vector.scalar_tensor_tensor(out=z3, in0=pm,
                                               scalar=b2_s, in1=zf,
                                               op0=ALU.add, op1=ALU.add)

                zo = io.tile([D, QC], F32, tag="zo")
                _ln_cols(nc, (work, ep2), z3, gpost_s, zo,
                         (ones_col, ones_colf), sr2)
                nc.sync.dma_start(outT[...], zo)

    nc.finalize()
    _CACHE["nc"] = nc
    return nc


def _prep_inputs(inputs):
    f32 = np.float32
    bf16 = ml_dtypes.bfloat16
    q = np.asarray(inputs["q"], f32)
    k = np.asarray(inputs["k"], f32)
    v = np.asarray(inputs["v"], f32)
    W = np.asarray(inputs["W_logits"], f32)
    vis = np.asarray(inputs["vis"]).astype(f32)
    skip = np.asarray(inputs["skip"], f32)

    g = lambda n: np.asarray(inputs[n], f32)
    qn_g, qn_b = g("qn_g"), g("qn_b")
    kn_g, kn_b = g("kn_g"), g("kn_b")
    vn_g, vn_b = g("vn_g"), g("vn_b")
    wq, bq = g("wq"), g("bq")
    wk, bk = g("wk"), g("bk")
    wv, bv = g("wv"), g("bv")
    wproj, bproj = g("wproj"), g("bproj")
    pre_g, pre_b = g("pre_g"), g("pre_b")
    w1, b1 = g("w1"), g("b1")
    w2, b2 = g("w2"), g("b2")
    post_g, post_b = g("post_g"), g("post_b")

    # fold LN affine into projections; fold attention scale into q path
    wq2 = (wq * qn_g[None, :]) * SCALE
    bq2 = (wq @ qn_b + bq) * SCALE
    wk2 = wk * kn_g[None, :]
    bk2 = wk @ kn_b + bk
    wv2 = wv * vn_g[None, :]
    bv2 = wv @ vn_b + bv

    def ln_rows(x):
        m = x.mean(-1, keepdims=True)
        var = x.var(-1, keepdims=True)
        return (x - m) / np.sqrt(var + EPS)

    # q -> normalized, transposed, padded [D, QPAD]
    qrows = q.reshape(D, Q).T
    qn = ln_rows(qrows)
    qTnp = np.zeros((D, QPAD), f32)
    qTnp[:, :Q] = qn.T
    skipTp = np.zeros((D, QPAD), f32)
    skipTp[:, :Q] = skip.reshape(D, Q)

    # k/v -> normalized rows, transposed [D, NKP] (pad cols zero)
    kRows = np.transpose(k, (0, 1, 3, 4, 2)).reshape(NK, D)
    vRows = np.transpose(v, (0, 1, 3, 4, 2)).reshape(NK, D)
    kTnp = np.zeros((D, NKP), f32)
    kTnp[:, :NK] = ln_rows(kRows).T
    vTnp = np.zeros((D, NKP), f32)
    vTnp[:, :NK] = ln_rows(vRows).T

    # combined mask W*vis (transposed, padded); vis for the first-order head
    Wp = np.zeros((QPAD, NKP), f32)
    Wp[:Q, :NK] = W[0] * vis[0]

    # wproj head-major: [inner, D] -> [DH, HEADS, D]
    wprojT = np.ascontiguousarray(wproj.T)
    wprojTm = np.ascontiguousarray(
        wprojT.reshape(HEADS, DH, D).transpose(1, 0, 2))

    wpackB = np.zeros((D, 1408), f32)
    wpackB[:, 0:128] = wq2.T
    wpackB[:, 128:256] = wk2.T
    wpackB[:, 256:384] = wv2.T
    wpackB[:, 384:640] = w1.T
    wpackB[:, 640:896] = w2.T.reshape(2, D, D).transpose(1, 0, 2).reshape(D, 256)
    wpackB[0:DH, 896:1408] = wprojTm.reshape(DH, HEADS * D)
    wpackF = np.zeros((D, 262), f32)
    wpackF[:, 0] = bq2
    wpackF[:, 1] = bk2
    wpackF[:, 2] = wproj @ bv2 + bproj
    wpackF[:, 3] = b2
    wpackF[:, 4:6] = b1.reshape(2, D).T
    wpackF[0, 6:134] = pre_g
    wpackF[1, 6:134] = pre_b
    wpackF[0, 134:262] = post_g
    wpackF[1, 134:262] = post_b
    shared = {
        "kTn": kTnp.astype(bf16),
        "vTn": vTnp.astype(bf16),
        "wpackB": wpackB.astype(bf16),
        "wpackF": wpackF,
    }

    in_maps = []
    for c in range(NCORES):
        sl = slice(c * QC, (c + 1) * QC)
        m = dict(shared)
        m["qTn"] = np.ascontiguousarray(qTnp[:, sl]).astype(bf16)
        m["skipT"] = np.ascontiguousarray(skipTp[:, sl])
        m["Wt"] = np.ascontiguousarray(
            Wp[sl].T.reshape(NKT, 128, QC).transpose(1, 0, 2)).astype(bf16)
        in_maps.append(m)
    return in_maps


def kernel(**inputs):
    from concourse.bass_utils import run_bass_kernel_spmd

    nc = _build()
    in_maps = _prep_inputs(inputs)
    res = run_bass_kernel_spmd(nc, in_maps, core_ids=list(range(NCORES)))
    outs = np.concatenate([r["outT"] for r in res.results], axis=1)  # [D, QPAD]
    return outs[:, :Q].reshape(1, D, HB, WB).astype(np.float32)


# revision 38
# speedup vs baseline: 1.4819x; 1.4819x over previous
"""CrossViewAttention Trainium2 kernel (v2).

Sharding: Q=2500 query positions across 8 cores (padded to 2560 = 8*320).
Softmax is over NK which stays local per core -> no collectives.

Per-core layout is fully "transposed": features on partitions, queries on
the free dim.  Host pre-normalizes q/k/v rows (LN folded into the
projection weights) and pre-multiplies W_logits*vis into a single mask.

Attention inner loop per nk-tile t (50 tiles of 128 keys):
  - 4 QK^T matmuls, one per head (contract dim 32), issued to distinct
    PE row groups via tile_position -> they run concurrently.
  - two DVE tensor_muls apply the combined W*vis mask (head pairs, so
    the single-buffered pl PSUM tiles free up early for the next QK).
  - one ACT exp() over all four heads [128, 1280].
  - 4 attn@V matmuls accumulate [33,320] per head; the 33rd row of V is
    ones and yields the softmax denominator for free.
QK^T for tile t+1 is emitted before attn@V for tile t so the PE FIFO
never stalls the DVE multiply chain.  A PE warm-up burst at the start
un-throttles the HAM clock gate during the input-DMA window.

Epilogue: softmax denominators are shifted to partition 0 by an
SBUF-to-SBUF DMA (the custom fast-reciprocal DVE op only works at
partition base 0), one reciprocal + per-head K=1 broadcast matmuls,
projection accumulate, skip add, LN -> MLP(gelu) -> LN.  LN uses
moment matmuls against a 1/128-scaled ones column, rstd = 1/sqrt via
ACT Sqrt + fast reciprocal, and the affine (g, b fold) is built as PE
outer products.  Activation functions are chosen so only the
exp/sqrt/gelu table sets are ever loaded (no ln/exp thrash).
"""

import sys

if "/opt/trn_rl_repo" not in sys.path:
    sys.path.insert(0, "/opt/trn_rl_repo")

import numpy as np
import ml_dtypes

import concourse.bass as bass
import concourse.bacc as bacc_mod
import concourse.mybir as mybir
from concourse.tile import TileContext

# problem constants (hardcoded per harness contract)
HEADS = 4
DH = 32
D = 128
EPS = 1e-5
HB = WB = 50
Q = HB * WB            # 2500
NVIEW, KH, KW = 6, 24, 44
NK = NVIEW * KH * KW   # 6336
NCORES = 8
QC = 320               # queries per core (Q padded to 2560)
QPAD = NCORES * QC
NKP = 6400             # NK padded to 50*128
NKT = NKP // 128       # 50 nk tiles
SCALE = DH ** -0.5

F32 = mybir.dt.float32
BF16 = mybir.dt.bfloat16
AF = mybir.ActivationFunctionType
ALU = mybir.AluOpType

_CACHE = {}


def _ln_cols(nc, pools, x, g2_lhsT, out, ones_cols, sr2):
    """LayerNorm of x [128, QC] f32 SBUF over the PARTITION dim.

    Stats come from ones-matmuls (ones scaled by 1/128 so the sums are the
    moments directly); rstd = exp(-0.5*ln(var+eps)); the affine
    (x - m)*rstd*g + b is applied as x*pA + pC where pA = g (x) rstd and
    pC = g (x) (-m*rstd) + b (x) 1 are built as PE outer products.
    """
    work, epi = pools
    ones_col, ones_colf = ones_cols
    sq = work.tile([D, QC], BF16, tag="lnsq")
    nc.scalar.activation(sq, x, AF.Square)
    s1 = epi.tile([1, QC], F32, tag="lns1")
    nc.tensor.matmul(s1, ones_colf, x, start=True, stop=True)
    s2 = epi.tile([1, QC], F32, tag="lns2")
    nc.tensor.matmul(s2, ones_col, sq, start=True, stop=True)
    ms = work.tile([1, QC], F32, tag="lnms")
    nc.scalar.activation(ms, s1, AF.Square)
    var = work.tile([1, QC], F32, tag="lnvar")
    nc.vector.tensor_tensor(out=var, in0=s2, in1=ms,
                            op=ALU.subtract)
    std = work.tile([1, QC], F32, tag="lnstd")
    nc.scalar.activation(std, var, AF.Sqrt, bias=EPS)
    rstd = work.tile([1, QC], F32, tag="lnrstd")
    nc.vector.reciprocal_approx_fast(out=rstd, in_=std)
    # sr2 row0 = -m*rstd, row1 = 1.0 (preset by caller)
    nc.vector.scalar_tensor_tensor(out=sr2[0:1, :], in0=s1,
                                   scalar=-1.0, in1=rstd,
                                   op0=ALU.mult, op1=ALU.mult)
    pA = epi.tile([D, QC], F32, tag="lnpA")
    nc.tensor.matmul(pA, g2_lhsT[0:1, :], rstd, start=True, stop=True)
    pC = epi.tile([D, QC], F32, tag="lnpC")
    nc.tensor.matmul(pC, g2_lhsT[0:2, :], sr2, start=True, stop=True)
    t1 = work.tile([D, QC], F32, tag="lnt1")
    nc.vector.tensor_mul(out=t1, in0=x, in1=pA)
    nc.vector.tensor_add(out=out, in0=t1, in1=pC)


def _build():
    if "nc" in _CACHE:
        return _CACHE["nc"]
    nc = bacc_mod.Bacc()

    # ---- I/O ----
    qTn = nc.dram_tensor("qTn", [D, QC], BF16, kind="ExternalInput")
    kTn = nc.dram_tensor("kTn", [D, NKP], BF16, kind="ExternalInput")
    vTn = nc.dram_tensor("vTn", [D, NKP], BF16, kind="ExternalInput")
    Wt = nc.dram_tensor("Wt", [128, NKT, QC], BF16, kind="ExternalInput")
    skipT = nc.dram_tensor("skipT", [D, QC], F32, kind="ExternalInput")
    # packed weights: [wq|wk|wv|w1(256)|w2(256)|wproj(512, rows 0:32)]
    wpackB = nc.dram_tensor("wpackB", [D, 1408], BF16, kind="ExternalInput")
    # packed f32 params: bq|bk|bproj|b2 (cols 0:4), b1m (4:6),
    # gpre (rows 0:2, cols 6:134), gpost (rows 0:2, cols 134:262)
    wpackF = nc.dram_tensor("wpackF", [D, 262], F32, kind="ExternalInput")
    outT = nc.dram_tensor("outT", [D, QC], F32, kind="ExternalOutput")

    with TileContext(nc) as tc:
        with tc.tile_pool(name="const", bufs=1) as cpool, \
             tc.tile_pool(name="big", bufs=1) as bigpool, \
             tc.tile_pool(name="work", bufs=3) as work, \
             tc.tile_pool(name="io", bufs=1) as io:

            # ---- constants ----
            zero_c = cpool.tile([128, 1], F32)
            nc.any.memset(zero_c, 0.0)
            nc.const_aps.aps[(F32, 0.0)] = zero_c[:]
            eps_c = cpool.tile([128, 1], F32)
            nc.any.memset(eps_c, EPS)
            nc.const_aps.aps[(F32, EPS)] = eps_c[:]
            ones_col = cpool.tile([128, 1], BF16)
            nc.any.memset(ones_col, 1.0 / 128.0)
            ones_colf = cpool.tile([128, 1], F32)
            nc.any.memset(ones_colf, 1.0 / 128.0)
            ones32bh = cpool.tile([1, 32], BF16)
            nc.any.memset(ones32bh, 1.0)
            identw = cpool.tile([D, D], BF16)
            nc.any.memset(identw, 0.5)

            wpB = cpool.tile([D, 1408], BF16)
            wpF = cpool.tile([D, 262], F32)
            wq_s = wpB[:, 0:128]
            wk_s = wpB[:, 128:256]
            wv_s = wpB[:, 256:384]
            w1_s = wpB[:, 384:640]
            w2_s = wpB[:, 640:896].rearrange("p (j d) -> p j d", j=2)
            wproj_s = wpB[0:DH, 896:1408].rearrange("p (h d) -> p h d",
                                                    h=HEADS)
            bq_s = wpF[:, 0:1]
            bk_s = wpF[:, 1:2]
            bproj_s = wpF[:, 2:3]
            b2_s = wpF[:, 3:4]
            b1_s = wpF[:, 4:6]
            gpre_s = wpF[0:2, 6:134]
            gpost_s = wpF[0:2, 134:262]

            # ---- resident tensors ----
            qTn_s = bigpool.tile([D, QC], BF16)
            skip_s = bigpool.tile([D, QC], F32)
            KCH = 512
            kTn_s = bigpool.tile([D, NKP], BF16)
            vTn_s = bigpool.tile([D, NKP], BF16)
            for c0 in range(0, NKP, 1600):
                nc.scalar.dma_start(kTn_s[:, c0:c0 + 1600],
                                    kTn[:, c0:c0 + 1600])
            for c0 in range(0, NKP, 1600):
                nc.gpsimd.dma_start(vTn_s[:, c0:c0 + 1600],
                                    vTn[:, c0:c0 + 1600])
            nc.scalar.dma_start(wpB, wpackB[...])
            nc.scalar.dma_start(wpF, wpackF[...])
            nc.scalar.dma_start(qTn_s, qTn[...])
            nc.gpsimd.dma_start(skip_s, skipT[...])

            kf = bigpool.tile([D, NKT, 128], BF16)
            vf = bigpool.tile([128, NKT, HEADS, DH + 1], BF16)
            qf = bigpool.tile([D, QC], BF16)
            Wsb = bigpool.tile([128, NKT, QC], BF16)
            WCH = 5
            for t0 in range(0, NKT, WCH):
                nc.sync.dma_start(Wsb[:, t0:t0 + WCH, :],
                                  Wt[:, t0:t0 + WCH, :])

            # ones column of V (softmax denominator); zero the k-padding rows
            nc.any.memset(vf[:, :, :, DH], 1.0)

            # ---- prep: projections ----
            with tc.tile_pool(name="psum_prep", bufs=2, space="PSUM") as ppre:
                # PE warm-up: dense matmuls from t~0 un-throttle the HAM
                # clock gate (1.2 -> 2.4 GHz) and bridge the input-DMA wait.
                for _ in range(30):
                    warm = ppre.tile([D, KCH], F32, tag="pk", name="warm")
                    nc.tensor.matmul(warm[:, :D], identw, identw, start=True,
                                     stop=True)
                pq = ppre.tile([D, KCH], F32, tag="pk", name="pq")
                nc.tensor.matmul(pq[:, :QC], wq_s, qTn_s, start=True,
                                 stop=True)
                nc.scalar.activation(qf, pq[:, :QC], AF.Identity, bias=bq_s)

                for i, c0 in enumerate(range(0, NKP, KCH)):
                    ce = min(c0 + KCH, NKP)
                    nt = (ce - c0) // 128
                    pk = ppre.tile([D, KCH], F32, tag="pk")
                    nc.tensor.matmul(pk[:, :ce - c0], wk_s, kTn_s[:, c0:ce],
                                     start=True, stop=True)
                    dst = kf[:, 4 * i:4 * i + nt, :]
                    if i % 2 == 0:
                        nc.scalar.activation(dst, pk[:, :ce - c0], AF.Identity,
                                             bias=bk_s)
                    else:
                        nc.vector.tensor_scalar(out=dst, in0=pk[:, :ce - c0],
                                                scalar1=bk_s,
                                                scalar2=None, op0=ALU.add)

                for i, t0 in enumerate(range(0, NKT, 4)):
                    nt = min(4, NKT - t0)
                    pv = ppre.tile([128, 4, 128], F32, tag="pv")
                    for j in range(nt):
                        t = t0 + j
                        nc.tensor.matmul(pv[:, j, :],
                                         vTn_s[:, t * 128:(t + 1) * 128],
                                         wv_s, start=True, stop=True)
                    src = pv[:, :nt, :].rearrange("p t (h e) -> p t h e",
                                                  h=HEADS)
                    dst = vf[:, t0:t0 + nt, :, :DH]
                    if i % 2 == 0:
                        nc.vector.tensor_copy(out=dst, in_=src)
                    else:
                        nc.scalar.activation(dst, src, AF.Identity)

                # zero v-values AND ones-row at the 64 padded key rows
                nc.any.memset(vf[64:128, NKT - 1, :, :], 0.0)

            # ---- attention ----
            # Software-pipelined: QK^T for t+1 is emitted BEFORE attn@V for
            # t so the PE FIFO never blocks the DVE multiply chain.
            with tc.tile_pool(name="psum_po", bufs=1, space="PSUM") as pop:
                po = [pop.tile([DH + 1, QC], F32, tag=f"po{h}",
                               name=f"po{h}")
                      for h in range(HEADS)]
                with tc.tile_pool(name="psum_pl", bufs=1, space="PSUM") as plp, \
                     tc.tile_pool(name="attw", bufs=2) as attw:
                    def qk(t):
                        plA = plp.tile([128, 2, 512], F32, tag="plA",
                                       name="plA")
                        plB = plp.tile([128, 2, 512], F32, tag="plB",
                                       name="plB")
                        for h in range(HEADS):
                            hb = 32 * h
                            dst = (plA, plB)[h // 2][:, h % 2, :QC]
                            nc.tensor.matmul(dst,
                                             kf[hb:hb + 32, t, :],
                                             qf[hb:hb + 32, :],
                                             start=True, stop=True,
                                             tile_position=(hb, 0))
                        return plA, plB

                    plA, plB = qk(0)
                    for t in range(NKT):
                        em = attw.tile([128, HEADS, QC], BF16, tag="em")
                        wbc = Wsb[:, t, None, :].to_broadcast((128, 2, QC))
                        ee = attw.tile([128, HEADS, QC], BF16, tag="ee")
                        nc.vector.tensor_mul(out=em[:, 0:2, :],
                                             in0=plA[:, :, :QC], in1=wbc)
                        nc.vector.tensor_mul(out=em[:, 2:4, :],
                                             in0=plB[:, :, :QC], in1=wbc)
                        nc.scalar.activation(ee, em, AF.Exp)
                        if t + 1 < NKT:
                            plA, plB = qk(t + 1)
                        for h in range(HEADS):
                            nc.tensor.matmul(po[h], vf[:, t, h, :],
                                             ee[:, h, :],
                                             start=(t == 0),
                                             stop=(t == NKT - 1))

                # ---- head normalize + projection accumulate ----
                with tc.tile_pool(name="psum_epi1", bufs=1, space="PSUM") as ep1:
                    den4 = io.tile([33, HEADS, QC], F32, tag="den4")
                    pz = ep1.tile([D, QC], F32, tag="pz")
                    for h in range(HEADS):
                        nc.scalar.activation(den4[DH:DH + 1, h, :],
                                             po[h][DH:DH + 1, :], AF.Copy)
                    # partition-shift 32 -> 0 via SBUF-to-SBUF DMA, then one
                    # fast reciprocal (the custom DVE op needs base 0)
                    den0 = io.tile([1, HEADS * QC], F32, tag="den0")
                    nc.sync.dma_start(
                        den0, den4[DH:DH + 1, :, :].rearrange("p h q -> p (h q)"))
                    rcp0 = io.tile([1, HEADS * QC], F32, tag="rcp0")
                    nc.vector.reciprocal_approx_fast(out=rcp0, in_=den0)
                    rcp0b = io.tile([1, HEADS * QC], BF16, tag="rcp0b")
                    nc.vector.tensor_copy(out=rcp0b, in_=rcp0)
                    rba = work.tile([DH, HEADS, QC], BF16, tag="rba")
                    for h in range(HEADS):
                        prh = ep1.tile([DH, 512], F32, tag="prh", bufs=2,
                                       name="prh")
                        nc.tensor.matmul(prh[:, :QC], ones32bh[0:1, :],
                                         rcp0b[:, h * QC:(h + 1) * QC],
                                         start=True, stop=True)
                        nc.scalar.activation(rba[:, h, :], prh[:, :QC],
                                             AF.Copy)
                    for h in range(HEADS):
                        onh = work.tile([DH, QC], BF16, tag="onh", name="onh")
                        nc.vector.tensor_mul(out=onh, in0=po[h][:DH, :],
                                             in1=rba[:, h, :])
                        nc.tensor.matmul(pz, wproj_s[:, h, :], onh,
                                         start=(h == 0),
                                         stop=(h == HEADS - 1))

                    # z = pz + bproj + skip
                    z = io.tile([D, QC], F32, tag="z")
                    nc.vector.scalar_tensor_tensor(out=z, in0=pz,
                                                   scalar=bproj_s,
                                                   in1=skip_s,
                                                   op0=ALU.add, op1=ALU.add)

            # ---- LN -> MLP -> LN ----
            with tc.tile_pool(name="psum_epi2", bufs=1, space="PSUM") as ep2:
                sr2 = io.tile([2, QC], F32, tag="sr2")
                nc.any.memset(sr2, 1.0)
                zf = io.tile([D, QC], F32, tag="zf")
                _ln_cols(nc, (work, ep2), z, gpre_s, zf,
                         (ones_col, ones_colf), sr2)
                zfb = io.tile([D, QC], BF16, tag="zfb")
                nc.vector.tensor_copy(out=zfb, in_=zf)

                h1 = io.tile([D, 2, QC], BF16, tag="h1")
                for j in range(2):
                    ph = ep2.tile([D, QC], F32, tag="ph", bufs=2)
                    nc.tensor.matmul(ph, w1_s[:, D * j:D * (j + 1)], zfb,
                                     start=True, stop=True)
                    nc.scalar.activation(h1[:, j, :], ph, AF.Gelu,
                                         bias=b1_s[:, j:j + 1])
                pm = ep2.tile([D, QC], F32, tag="pm")
                nc.tensor.matmul(pm, w2_s[:, 0, :], h1[:, 0, :],
                                 start=True, stop=False)
                nc.tensor.matmul(pm, w2_s[:, 1, :], h1[:, 1, :],
                                 start=False, stop=True)
                z3 = io.tile([D, QC], F32, tag="z3")
                nc.vector.scalar_tensor_tensor(out=z3, in0=pm,
                                               scalar=b2_s, in1=zf,
                                               op0=ALU.add, op1=ALU.add)

                zo = io.tile([D, QC], F32, tag="zo")
                _ln_cols(nc, (work, ep2), z3, gpost_s, zo,
                         (ones_col, ones_colf), sr2)
                nc.sync.dma_start(outT[...], zo)

    nc.finalize()
    _CACHE["nc"] = nc
    return nc


def _prep_inputs(inputs):
    f32 = np.float32
    bf16 = ml_dtypes.bfloat16
    q = np.asarray(inputs["q"], f32)
    k = np.asarray(inputs["k"], f32)
    v = np.asarray(inputs["v"], f32)
    W = np.asarray(inputs["W_logits"], f32)
    vis = np.asarray(inputs["vis"]).astype(f32)
    skip = np.asarray(inputs["skip"], f32)

    g = lambda n: np.asarray(inputs[n], f32)
    qn_g, qn_b = g("qn_g"), g("qn_b")
    kn_g, kn_b = g("kn_g"), g("kn_b")
    vn_g, vn_b = g("vn_g"), g("vn_b")
    wq, bq = g("wq"), g("bq")
    wk, bk = g("wk"), g("bk")
    wv, bv = g("wv"), g("bv")
    wproj, bproj = g("wproj"), g("bproj")
    pre_g, pre_b = g("pre_g"), g("pre_b")
    w1, b1 = g("w1"), g("b1")
    w2, b2 = g("w2"), g("b2")
    post_g, post_b = g("post_g"), g("post_b")

    # fold LN affine into projections; fold attention scale into q path
    wq2 = (wq * qn_g[None, :]) * SCALE
    bq2 = (wq @ qn_b + bq) * SCALE
    wk2 = wk * kn_g[None, :]
    bk2 = wk @ kn_b + bk
    wv2 = wv * vn_g[None, :]
    bv2 = wv @ vn_b + bv

    def ln_rows(x):
        m = x.mean(-1, keepdims=True)
        var = x.var(-1, keepdims=True)
        return (x - m) / np.sqrt(var + EPS)

    # q -> normalized, transposed, padded [D, QPAD]
    qrows = q.reshape(D, Q).T
    qn = ln_rows(qrows)
    qTnp = np.zeros((D, QPAD), f32)
    qTnp[:, :Q] = qn.T
    skipTp = np.zeros((D, QPAD), f32)
    skipTp[:, :Q] = skip.reshape(D, Q)

    # k/v -> normalized rows, transposed [D, NKP] (pad cols zero)
    kRows = np.transpose(k, (0, 1, 3, 4, 2)).reshape(NK, D)
    vRows = np.transpose(v, (0, 1, 3, 4, 2)).reshape(NK, D)
    kTnp = np.zeros((D, NKP), f32)
    kTnp[:, :NK] = ln_rows(kRows).T
    vTnp = np.zeros((D, NKP), f32)
    vTnp[:, :NK] = ln_rows(vRows).T

    # combined mask W*vis (transposed, padded); vis for the first-order head
    Wp = np.zeros((QPAD, NKP), f32)
    Wp[:Q, :NK] = W[0] * vis[0]

    # wproj head-major: [inner, D] -> [DH, HEADS, D]
    wprojT = np.ascontiguousarray(wproj.T)
    wprojTm = np.ascontiguousarray(
        wprojT.reshape(HEADS, DH, D).transpose(1, 0, 2))

    wpackB = np.zeros((D, 1408), f32)
    wpackB[:, 0:128] = wq2.T
    wpackB[:, 128:256] = wk2.T
    wpackB[:, 256:384] = wv2.T
    wpackB[:, 384:640] = w1.T
    wpackB[:, 640:896] = w2.T.reshape(2, D, D).transpose(1, 0, 2).reshape(D, 256)
    wpackB[0:DH, 896:1408] = wprojTm.reshape(DH, HEADS * D)
    wpackF = np.zeros((D, 262), f32)
    wpackF[:, 0] = bq2
    wpackF[:, 1] = bk2
    wpackF[:, 2] = wproj @ bv2 + bproj
    wpackF[:, 3] = b2
    wpackF[:, 4:6] = b1.reshape(2, D).T
    wpackF[0, 6:134] = pre_g
    wpackF[1, 6:134] = pre_b
    wpackF[0, 134:262] = post_g
    wpackF[1, 134:262] = post_b
    shared = {
        "kTn": kTnp.astype(bf16),
        "vTn": vTnp.astype(bf16),
        "wpackB": wpackB.astype(bf16),
        "wpackF": wpackF,
    }

    in_maps = []
    for c in range(NCORES):
        sl = slice(c * QC, (c + 1) * QC)
        m = dict(shared)
        m["qTn"] = np.ascontiguousarray(qTnp[:, sl]).astype(bf16)
        m["skipT"] = np.ascontiguousarray(skipTp[:, sl])
        m["Wt"] = np.ascontiguousarray(
            Wp[sl].T.reshape(NKT, 128, QC).transpose(1, 0, 2)).astype(bf16)
        in_maps.append(m)
    return in_maps


def kernel(**inputs):
    from concourse.bass_utils import run_bass_kernel_spmd

    nc = _build()
    in_maps = _prep_inputs(inputs)
    res = run_bass_kernel_spmd(nc, in_maps, core_ids=list(range(NCORES)))
    outs = np.concatenate([r["outT"] for r in res.results], axis=1)  # [D, QPAD]
    return outs[:, :Q].reshape(1, D, HB, WB).astype(np.float32)


# revision 39
# speedup vs baseline: 1.5471x; 1.0440x over previous
"""CrossViewAttention Trainium2 kernel (v2).

Sharding: Q=2500 query positions across 8 cores (padded to 2560 = 8*320).
Softmax is over NK which stays local per core -> no collectives.

Per-core layout is fully "transposed": features on partitions, queries on
the free dim.  Host pre-normalizes q/k/v rows (LN folded into the
projection weights) and pre-multiplies W_logits*vis into a single mask.

Attention inner loop per nk-tile t (50 tiles of 128 keys):
  - 4 QK^T matmuls, one per head (contract dim 32), issued to distinct
    PE row groups via tile_position -> they run concurrently.
  - two DVE tensor_muls apply the combined W*vis mask (head pairs, so
    the single-buffered pl PSUM tiles free up early for the next QK).
  - one ACT exp() over all four heads [128, 1280].
  - 4 attn@V matmuls accumulate [33,320] per head; the 33rd row of V is
    ones and yields the softmax denominator for free.
QK^T for tile t+1 is emitted before attn@V for tile t so the PE FIFO
never stalls the DVE multiply chain.  A PE warm-up burst at the start
un-throttles the HAM clock gate during the input-DMA window.

Epilogue: softmax denominators are shifted to partition 0 by an
SBUF-to-SBUF DMA (the custom fast-reciprocal DVE op only works at
partition base 0), one reciprocal + per-head K=1 broadcast matmuls,
projection accumulate, skip add, LN -> MLP(gelu) -> LN.  LN uses
moment matmuls against a 1/128-scaled ones column, rstd = 1/sqrt via
ACT Sqrt + fast reciprocal, and the affine (g, b fold) is built as PE
outer products.  Activation functions are chosen so only the
exp/sqrt/gelu table sets are ever loaded (no ln/exp thrash).
"""

import sys

if "/opt/trn_rl_repo" not in sys.path:
    sys.path.insert(0, "/opt/trn_rl_repo")

import numpy as np
import ml_dtypes

import concourse.bass as bass
import concourse.bacc as bacc_mod
import concourse.mybir as mybir
from concourse.tile import TileContext

# problem constants (hardcoded per harness contract)
HEADS = 4
DH = 32
D = 128
EPS = 1e-5
HB = WB = 50
Q = HB * WB            # 2500
NVIEW, KH, KW = 6, 24, 44
NK = NVIEW * KH * KW   # 6336
NCORES = 8
QC = 320               # queries per core (Q padded to 2560)
QPAD = NCORES * QC
NKP = 6400             # NK padded to 50*128
NKT = NKP // 128       # 50 nk tiles
SCALE = DH ** -0.5

F32 = mybir.dt.float32
BF16 = mybir.dt.bfloat16
AF = mybir.ActivationFunctionType
ALU = mybir.AluOpType

_CACHE = {}


def _ln_cols(nc, pools, x, g2_lhsT, out, ones_cols, sr2):
    """LayerNorm of x [128, QC] f32 SBUF over the PARTITION dim.

    Stats come from ones-matmuls (ones scaled by 1/128 so the sums are the
    moments directly); rstd = exp(-0.5*ln(var+eps)); the affine
    (x - m)*rstd*g + b is applied as x*pA + pC where pA = g (x) rstd and
    pC = g (x) (-m*rstd) + b (x) 1 are built as PE outer products.
    """
    work, epi = pools
    ones_col, ones_colf = ones_cols
    sq = work.tile([D, QC], BF16, tag="lnsq")
    nc.scalar.activation(sq, x, AF.Square)
    s1 = epi.tile([1, QC], F32, tag="lns1")
    nc.tensor.matmul(s1, ones_colf, x, start=True, stop=True)
    s2 = epi.tile([1, QC], F32, tag="lns2")
    nc.tensor.matmul(s2, ones_col, sq, start=True, stop=True)
    ms = work.tile([1, QC], F32, tag="lnms")
    nc.scalar.activation(ms, s1, AF.Square)
    var = work.tile([1, QC], F32, tag="lnvar")
    nc.vector.tensor_tensor(out=var, in0=s2, in1=ms,
                            op=ALU.subtract)
    std = work.tile([1, QC], F32, tag="lnstd")
    nc.scalar.activation(std, var, AF.Sqrt, bias=EPS)
    rstd = work.tile([1, QC], F32, tag="lnrstd")
    nc.vector.reciprocal_approx_fast(out=rstd, in_=std)
    # sr2 row0 = -m*rstd, row1 = 1.0 (preset by caller)
    nc.vector.scalar_tensor_tensor(out=sr2[0:1, :], in0=s1,
                                   scalar=-1.0, in1=rstd,
                                   op0=ALU.mult, op1=ALU.mult)
    pA = epi.tile([D, QC], F32, tag="lnpA")
    nc.tensor.matmul(pA, g2_lhsT[0:1, :], rstd, start=True, stop=True)
    pC = epi.tile([D, QC], F32, tag="lnpC")
    nc.tensor.matmul(pC, g2_lhsT[0:2, :], sr2, start=True, stop=True)
    t1 = work.tile([D, QC], F32, tag="lnt1")
    nc.vector.tensor_mul(out=t1, in0=x, in1=pA)
    nc.vector.tensor_add(out=out, in0=t1, in1=pC)


def _build():
    if "nc" in _CACHE:
        return _CACHE["nc"]
    nc = bacc_mod.Bacc()

    # ---- I/O ----
    qTn = nc.dram_tensor("qTn", [D, QC], BF16, kind="ExternalInput")
    kTn = nc.dram_tensor("kTn", [D, NKP], BF16, kind="ExternalInput")
    vTn = nc.dram_tensor("vTn", [D, NKP], BF16, kind="ExternalInput")
    Wt = nc.dram_tensor("Wt", [128, NKT, QC], BF16, kind="ExternalInput")
    skipT = nc.dram_tensor("skipT", [D, QC], F32, kind="ExternalInput")
    # packed weights: [wq|wk|wv|w1(256)|w2(256)|wproj(512, rows 0:32)]
    wpackB = nc.dram_tensor("wpackB", [D, 1408], BF16, kind="ExternalInput")
    # packed f32 params: bq|bk|bproj|b2 (cols 0:4), b1m (4:6),
    # gpre (rows 0:2, cols 6:134), gpost (rows 0:2, cols 134:262)
    wpackF = nc.dram_tensor("wpackF", [D, 262], F32, kind="ExternalInput")
    outT = nc.dram_tensor("outT", [D, QC], F32, kind="ExternalOutput")

    with TileContext(nc) as tc:
        with tc.tile_pool(name="const", bufs=1) as cpool, \
             tc.tile_pool(name="big", bufs=1) as bigpool, \
             tc.tile_pool(name="work", bufs=3) as work, \
             tc.tile_pool(name="io", bufs=1) as io:

            # ---- constants ----
            zero_c = cpool.tile([128, 1], F32)
            nc.any.memset(zero_c, 0.0)
            nc.const_aps.aps[(F32, 0.0)] = zero_c[:]
            eps_c = cpool.tile([128, 1], F32)
            nc.any.memset(eps_c, EPS)
            nc.const_aps.aps[(F32, EPS)] = eps_c[:]
            ones_col = cpool.tile([128, 1], BF16)
            nc.any.memset(ones_col, 1.0 / 128.0)
            ones_colf = cpool.tile([128, 1], F32)
            nc.any.memset(ones_colf, 1.0 / 128.0)
            ones32bh = cpool.tile([1, 32], BF16)
            nc.any.memset(ones32bh, 1.0)
            identw = cpool.tile([D, D], BF16)
            nc.any.memset(identw, 0.5)

            wpB = cpool.tile([D, 1408], BF16)
            wpF = cpool.tile([D, 262], F32)
            wq_s = wpB[:, 0:128]
            wk_s = wpB[:, 128:256]
            wv_s = wpB[:, 256:384]
            w1_s = wpB[:, 384:640]
            w2_s = wpB[:, 640:896].rearrange("p (j d) -> p j d", j=2)
            wproj_s = wpB[0:DH, 896:1408].rearrange("p (h d) -> p h d",
                                                    h=HEADS)
            bq_s = wpF[:, 0:1]
            bk_s = wpF[:, 1:2]
            bproj_s = wpF[:, 2:3]
            b2_s = wpF[:, 3:4]
            b1_s = wpF[:, 4:6]
            gpre_s = wpF[0:2, 6:134]
            gpost_s = wpF[0:2, 134:262]

            # ---- resident tensors ----
            qTn_s = bigpool.tile([D, QC], BF16)
            skip_s = bigpool.tile([D, QC], F32)
            KCH = 512
            kTn_s = bigpool.tile([D, NKP], BF16)
            vTn_s = bigpool.tile([D, NKP], BF16)
            for c0 in range(0, NKP, 1600):
                nc.scalar.dma_start(kTn_s[:, c0:c0 + 1600],
                                    kTn[:, c0:c0 + 1600])
            for c0 in range(0, NKP, 1600):
                nc.gpsimd.dma_start(vTn_s[:, c0:c0 + 1600],
                                    vTn[:, c0:c0 + 1600])
            nc.scalar.dma_start(wpB, wpackB[...])
            nc.scalar.dma_start(wpF, wpackF[...])
            nc.scalar.dma_start(qTn_s, qTn[...])
            nc.gpsimd.dma_start(skip_s, skipT[...])

            kf = bigpool.tile([D, NKT, 128], BF16)
            vf = bigpool.tile([128, NKT, HEADS, DH + 1], BF16)
            qf = bigpool.tile([D, QC], BF16)
            Wsb = bigpool.tile([128, NKT, QC], BF16)
            WCH = 5
            for t0 in range(0, NKT, WCH):
                nc.sync.dma_start(Wsb[:, t0:t0 + WCH, :],
                                  Wt[:, t0:t0 + WCH, :])

            # ones column of V (softmax denominator); zero the k-padding rows
            nc.any.memset(vf[:, :, :, DH], 1.0)

            # ---- prep: projections ----
            with tc.tile_pool(name="psum_prep", bufs=2, space="PSUM") as ppre:
                # PE warm-up: dense matmuls from t~0 un-throttle the HAM
                # clock gate (1.2 -> 2.4 GHz) and bridge the input-DMA wait.
                for _ in range(45):
                    warm = ppre.tile([D, KCH], F32, tag="pk", name="warm")
                    nc.tensor.matmul(warm[:, :D], identw, identw, start=True,
                                     stop=True)
                pq = ppre.tile([D, KCH], F32, tag="pk", name="pq")
                nc.tensor.matmul(pq[:, :QC], wq_s, qTn_s, start=True,
                                 stop=True)
                nc.scalar.activation(qf, pq[:, :QC], AF.Identity, bias=bq_s)

                for i, c0 in enumerate(range(0, NKP, KCH)):
                    ce = min(c0 + KCH, NKP)
                    nt = (ce - c0) // 128
                    pk = ppre.tile([D, KCH], F32, tag="pk")
                    nc.tensor.matmul(pk[:, :ce - c0], wk_s, kTn_s[:, c0:ce],
                                     start=True, stop=True)
                    dst = kf[:, 4 * i:4 * i + nt, :]
                    if i % 2 == 0:
                        nc.scalar.activation(dst, pk[:, :ce - c0], AF.Identity,
                                             bias=bk_s)
                    else:
                        nc.vector.tensor_scalar(out=dst, in0=pk[:, :ce - c0],
                                                scalar1=bk_s,
                                                scalar2=None, op0=ALU.add)

                for i, t0 in enumerate(range(0, NKT, 4)):
                    nt = min(4, NKT - t0)
                    pv = ppre.tile([128, 4, 128], F32, tag="pv")
                    for j in range(nt):
                        t = t0 + j
                        nc.tensor.matmul(pv[:, j, :],
                                         vTn_s[:, t * 128:(t + 1) * 128],
                                         wv_s, start=True, stop=True)
                    src = pv[:, :nt, :].rearrange("p t (h e) -> p t h e",
                                                  h=HEADS)
                    dst = vf[:, t0:t0 + nt, :, :DH]
                    if i % 2 == 0:
                        nc.vector.tensor_copy(out=dst, in_=src)
                    else:
                        nc.scalar.activation(dst, src, AF.Identity)

                # zero v-values AND ones-row at the 64 padded key rows
                nc.any.memset(vf[64:128, NKT - 1, :, :], 0.0)

            # ---- attention ----
            # Software-pipelined: QK^T for t+1 is emitted BEFORE attn@V for
            # t so the PE FIFO never blocks the DVE multiply chain.
            with tc.tile_pool(name="psum_po", bufs=1, space="PSUM") as pop:
                po = [pop.tile([DH + 1, QC], F32, tag=f"po{h}",
                               name=f"po{h}")
                      for h in range(HEADS)]
                with tc.tile_pool(name="psum_pl", bufs=1, space="PSUM") as plp, \
                     tc.tile_pool(name="attw", bufs=3) as attw:
                    def qk(t):
                        plA = plp.tile([128, 2, 512], F32, tag="plA",
                                       name="plA")
                        plB = plp.tile([128, 2, 512], F32, tag="plB",
                                       name="plB")
                        for h in range(HEADS):
                            hb = 32 * h
                            dst = (plA, plB)[h // 2][:, h % 2, :QC]
                            nc.tensor.matmul(dst,
                                             kf[hb:hb + 32, t, :],
                                             qf[hb:hb + 32, :],
                                             start=True, stop=True,
                                             tile_position=(hb, 0))
                        return plA, plB

                    plA, plB = qk(0)
                    for t in range(NKT):
                        em = attw.tile([128, HEADS, QC], BF16, tag="em")
                        wbc = Wsb[:, t, None, :].to_broadcast((128, 2, QC))
                        ee = attw.tile([128, HEADS, QC], BF16, tag="ee")
                        nc.vector.tensor_mul(out=em[:, 0:2, :],
                                             in0=plA[:, :, :QC], in1=wbc)
                        nc.vector.tensor_mul(out=em[:, 2:4, :],
                                             in0=plB[:, :, :QC], in1=wbc)
                        nc.scalar.activation(ee, em, AF.Exp)
                        if t + 1 < NKT:
                            plA, plB = qk(t + 1)
                        for h in range(HEADS):
                            nc.tensor.matmul(po[h], vf[:, t, h, :],
                                             ee[:, h, :],
                                             start=(t == 0),
                                             stop=(t == NKT - 1))

                # ---- head normalize + projection accumulate ----
                with tc.tile_pool(name="psum_epi1", bufs=1, space="PSUM") as ep1:
                    den4 = io.tile([33, HEADS, QC], F32, tag="den4")
                    pz = ep1.tile([D, QC], F32, tag="pz")
                    for h in range(HEADS):
                        if h % 2 == 0:
                            nc.scalar.activation(den4[DH:DH + 1, h, :],
                                                 po[h][DH:DH + 1, :],
                                                 AF.Copy)
                        else:
                            nc.vector.tensor_copy(
                                out=den4[DH:DH + 1, h, :],
                                in_=po[h][DH:DH + 1, :])
                    # partition-shift 32 -> 0 via SBUF-to-SBUF DMA, then one
                    # fast reciprocal (the custom DVE op needs base 0)
                    den0 = io.tile([1, HEADS * QC], F32, tag="den0")
                    nc.sync.dma_start(
                        den0, den4[DH:DH + 1, :, :].rearrange("p h q -> p (h q)"))
                    rcp0 = io.tile([1, HEADS * QC], F32, tag="rcp0")
                    nc.vector.reciprocal_approx_fast(out=rcp0, in_=den0)
                    rcp0b = io.tile([1, HEADS * QC], BF16, tag="rcp0b")
                    nc.vector.tensor_copy(out=rcp0b, in_=rcp0)
                    rba = work.tile([DH, HEADS, QC], BF16, tag="rba")
                    for h in range(HEADS):
                        prh = ep1.tile([DH, 512], F32, tag="prh", bufs=2,
                                       name="prh")
                        nc.tensor.matmul(prh[:, :QC], ones32bh[0:1, :],
                                         rcp0b[:, h * QC:(h + 1) * QC],
                                         start=True, stop=True)
                        nc.scalar.activation(rba[:, h, :], prh[:, :QC],
                                             AF.Copy)
                    for h in range(HEADS):
                        onh = work.tile([DH, QC], BF16, tag="onh", name="onh")
                        nc.vector.tensor_mul(out=onh, in0=po[h][:DH, :],
                                             in1=rba[:, h, :])
                        nc.tensor.matmul(pz, wproj_s[:, h, :], onh,
                                         start=(h == 0),
                                         stop=(h == HEADS - 1))

                    # z = pz + bproj + skip
                    z = io.tile([D, QC], F32, tag="z")
                    nc.vector.scalar_tensor_tensor(out=z, in0=pz,
                                                   scalar=bproj_s,
                                                   in1=skip_s,
                                                   op0=ALU.add, op1=ALU.add)

            # ---- LN -> MLP -> LN ----
            with tc.tile_pool(name="psum_epi2", bufs=1, space="PSUM") as ep2:
                sr2 = io.tile([2, QC], F32, tag="sr2")
                nc.any.memset(sr2, 1.0)
                zf = io.tile([D, QC], F32, tag="zf")
                _ln_cols(nc, (work, ep2), z, gpre_s, zf,
                         (ones_col, ones_colf), sr2)
                zfb = io.tile([D, QC], BF16, tag="zfb")
                nc.vector.tensor_copy(out=zfb, in_=zf)

                h1 = io.tile([D, 2, QC], BF16, tag="h1")
                for j in range(2):
                    ph = ep2.tile([D, QC], F32, tag="ph", bufs=2)
                    nc.tensor.matmul(ph, w1_s[:, D * j:D * (j + 1)], zfb,
                                     start=True, stop=True)
                    nc.scalar.activation(h1[:, j, :], ph, AF.Gelu,
                                         bias=b1_s[:, j:j + 1])
                pm = ep2.tile([D, QC], F32, tag="pm")
                nc.tensor.matmul(pm, w2_s[:, 0, :], h1[:, 0, :],
                                 start=True, stop=False)
                nc.tensor.matmul(pm, w2_s[:, 1, :], h1[:, 1, :],
                                 start=False, stop=True)
                z3 = io.tile([D, QC], F32, tag="z3")
                nc.vector.scalar_tensor_tensor(out=z3, in0=pm,
                                               scalar=b2_s, in1=zf,
                                               op0=ALU.add, op1=ALU.add)

                zo = io.tile([D, QC], F32, tag="zo")
                _ln_cols(nc, (work, ep2), z3, gpost_s, zo,
                         (ones_col, ones_colf), sr2)
                nc.sync.dma_start(outT[...], zo)

    nc.finalize()
    _CACHE["nc"] = nc
    return nc


def _prep_inputs(inputs):
    f32 = np.float32
    bf16 = ml_dtypes.bfloat16
    q = np.asarray(inputs["q"], f32)
    k = np.asarray(inputs["k"], f32)
    v = np.asarray(inputs["v"], f32)
    W = np.asarray(inputs["W_logits"], f32)
    vis = np.asarray(inputs["vis"]).astype(f32)
    skip = np.asarray(inputs["skip"], f32)

    g = lambda n: np.asarray(inputs[n], f32)
    qn_g, qn_b = g("qn_g"), g("qn_b")
    kn_g, kn_b = g("kn_g"), g("kn_b")
    vn_g, vn_b = g("vn_g"), g("vn_b")
    wq, bq = g("wq"), g("bq")
    wk, bk = g("wk"), g("bk")
    wv, bv = g("wv"), g("bv")
    wproj, bproj = g("wproj"), g("bproj")
    pre_g, pre_b = g("pre_g"), g("pre_b")
    w1, b1 = g("w1"), g("b1")
    w2, b2 = g("w2"), g("b2")
    post_g, post_b = g("post_g"), g("post_b")

    # fold LN affine into projections; fold attention scale into q path
    wq2 = (wq * qn_g[None, :]) * SCALE
    bq2 = (wq @ qn_b + bq) * SCALE
    wk2 = wk * kn_g[None, :]
    bk2 = wk @ kn_b + bk
    wv2 = wv * vn_g[None, :]
    bv2 = wv @ vn_b + bv

    def ln_rows(x):
        m = x.mean(-1, keepdims=True)
        var = x.var(-1, keepdims=True)
        return (x - m) / np.sqrt(var + EPS)

    # q -> normalized, transposed, padded [D, QPAD]
    qrows = q.reshape(D, Q).T
    qn = ln_rows(qrows)
    qTnp = np.zeros((D, QPAD), f32)
    qTnp[:, :Q] = qn.T
    skipTp = np.zeros((D, QPAD), f32)
    skipTp[:, :Q] = skip.reshape(D, Q)

    # k/v -> normalized rows, transposed [D, NKP] (pad cols zero)
    kRows = np.transpose(k, (0, 1, 3, 4, 2)).reshape(NK, D)
    vRows = np.transpose(v, (0, 1, 3, 4, 2)).reshape(NK, D)
    kTnp = np.zeros((D, NKP), f32)
    kTnp[:, :NK] = ln_rows(kRows).T
    vTnp = np.zeros((D, NKP), f32)
    vTnp[:, :NK] = ln_rows(vRows).T

    # combined mask W*vis (transposed, padded); vis for the first-order head
    Wp = np.zeros((QPAD, NKP), f32)
    Wp[:Q, :NK] = W[0] * vis[0]

    # wproj head-major: [inner, D] -> [DH, HEADS, D]
    wprojT = np.ascontiguousarray(wproj.T)
    wprojTm = np.ascontiguousarray(
        wprojT.reshape(HEADS, DH, D).transpose(1, 0, 2))

    wpackB = np.zeros((D, 1408), f32)
    wpackB[:, 0:128] = wq2.T
    wpackB[:, 128:256] = wk2.T
    wpackB[:, 256:384] = wv2.T
    wpackB[:, 384:640] = w1.T
    wpackB[:, 640:896] = w2.T.reshape(2, D, D).transpose(1, 0, 2).reshape(D, 256)
    wpackB[0:DH, 896:1408] = wprojTm.reshape(DH, HEADS * D)
    wpackF = np.zeros((D, 262), f32)
    wpackF[:, 0] = bq2
    wpackF[:, 1] = bk2
    wpackF[:, 2] = wproj @ bv2 + bproj
    wpackF[:, 3] = b2
    wpackF[:, 4:6] = b1.reshape(2, D).T
    wpackF[0, 6:134] = pre_g
    wpackF[1, 6:134] = pre_b
    wpackF[0, 134:262] = post_g
    wpackF[1, 134:262] = post_b
    shared = {
        "kTn": kTnp.astype(bf16),
        "vTn": vTnp.astype(bf16),
        "wpackB": wpackB.astype(bf16),
        "wpackF": wpackF,
    }

    in_maps = []
    for c in range(NCORES):
        sl = slice(c * QC, (c + 1) * QC)
        m = dict(shared)
        m["qTn"] = np.ascontiguousarray(qTnp[:, sl]).astype(bf16)
        m["skipT"] = np.ascontiguousarray(skipTp[:, sl])
        m["Wt"] = np.ascontiguousarray(
            Wp[sl].T.reshape(NKT, 128, QC).transpose(1, 0, 2)).astype(bf16)
        in_maps.append(m)
    return in_maps


def kernel(**inputs):
    from concourse.bass_utils import run_bass_kernel_spmd

    nc = _build()
    in_maps = _prep_inputs(inputs)
    res = run_bass_kernel_spmd(nc, in_maps, core_ids=list(range(NCORES)))
    outs = np.concatenate([r["outT"] for r in res.results], axis=1)  # [D, QPAD]
    return outs[:, :Q].reshape(1, D, HB, WB).astype(np.float32)
